# revision 1
# baseline (speedup 1.0000x reference)
"""MithralNN (PQ/vq_codebook) Trainium2 kernel.

out[n, m] = sum_c luts[c, argmin_k ||X[n, cS:(c+1)S] - protos[c,k]||^2, m] * mean(X)

Strategy (8 NeuronCores, data-parallel over rows of X):
  host:   transpose each core's X shard to [D, NL] (contiguous DMA of X^T),
          build block-diagonal -2*protos matmul weights, pre-broadcast
          ||p||^2 bias, pre-scale luts by mean(X).
  device: dists = X^T-block-matmuls + bias   (PE, contraction over d)
          one-hot(argmin_k) via min-reduce + is_equal   (DVE)
          onehot^T via PE transpose, out = onehot @ luts via PE matmuls
          PSUM->SBUF copies split across ScalarE/VectorE, DMA out.
"""

import os
import sys
import types

import numpy as np

import concourse.bacc as bacc
import concourse.mybir as mybir
import concourse.tile as tile
from concourse.bass_utils import run_bass_kernel_spmd

F32 = mybir.dt.float32

N, D = 32768, 512
C, K, S = 16, 16, 32
CK = C * K          # 256
M = 1024
NCORES = 8
NL = N // NCORES    # 4096 rows per core
NT = NL // 128      # 32 row tiles per core


def _install_profile_shim():
    """Optionally enable NTFF profiling through the axon tunnel (dev only)."""
    try:
        if "antenv.axon_hooks" not in sys.modules:
            import antenv

            mod = types.ModuleType("antenv.axon_hooks")
            mod._hook = None
            mod.set_axon_ntff_profile_hook = lambda h: setattr(mod, "_hook", h)
            mod.get_axon_ntff_profile_hook = lambda: mod._hook
            sys.modules["antenv.axon_hooks"] = mod
            antenv.axon_hooks = mod
            from trn_agent_boot.trn_boot import _ntff_profile_via_ctypes

            mod.set_axon_ntff_profile_hook(
                _ntff_profile_via_ctypes("/opt/axon/libaxon_pjrt.so")
            )
        return True
    except Exception:
        return False


def _build_program():
    nc = bacc.Bacc("TRN2", target_bir_lowering=False, debug=False)

    xt = nc.dram_tensor("xt", [D, NL], F32, kind="ExternalInput")
    w4 = nc.dram_tensor("w4", [128, 256], F32, kind="ExternalInput")
    pnb = nc.dram_tensor("pnb", [128, CK], F32, kind="ExternalInput")
    idm = nc.dram_tensor("idm", [128, 128], F32, kind="ExternalInput")
    luts = nc.dram_tensor("luts", [128, 2 * M], F32, kind="ExternalInput")
    out = nc.dram_tensor("out", [NL, M], F32, kind="ExternalOutput")

    add = mybir.AluOpType.add
    amin = mybir.AluOpType.min
    iseq = mybir.AluOpType.is_equal
    ax_x = mybir.AxisListType.X

    with tile.TileContext(nc) as tc:
        with (
            tc.tile_pool(name="const", bufs=1) as constp,
            tc.tile_pool(name="xch", bufs=3) as xpool,
            tc.tile_pool(name="work", bufs=3) as workp,
            tc.tile_pool(name="osb", bufs=3) as osbp,
            tc.tile_pool(name="pdst", bufs=2, space="PSUM") as pd,
            tc.tile_pool(name="ptr", bufs=2, space="PSUM") as pt,
            tc.tile_pool(name="pout", bufs=2, space="PSUM") as po,
        ):
            w4sb = constp.tile([128, 256], F32)
            nc.sync.dma_start(w4sb[:], w4[:])
            pnsb = constp.tile([128, CK], F32)
            nc.sync.dma_start(pnsb[:], pnb[:])
            idsb = constp.tile([128, 128], F32)
            nc.sync.dma_start(idsb[:], idm[:])
            lutsb = constp.tile([128, 2 * M], F32)
            nc.sync.dma_start(lutsb[:], luts[:])

            for j in range(NT // 4):          # 8 chunks of 512 rows
                cj = xpool.tile([128, 4 * 512], F32, tag="xchunk")
                for b in range(4):
                    nc.sync.dma_start(
                        cj[:, 512 * b : 512 * (b + 1)],
                        xt[128 * b : 128 * (b + 1), 512 * j : 512 * (j + 1)],
                    )
                for tt in range(4):
                    t = 4 * j + tt
                    # --- encode: dists'[n, ck] = sum_d xt[d,n] * (-2W)[d,ck]
                    dps = pd.tile([128, CK], F32, tag="dps")
                    for b in range(4):
                        nc.tensor.matmul(
                            dps[:, 64 * b : 64 * (b + 1)],
                            lhsT=cj[:, 512 * b + 128 * tt : 512 * b + 128 * (tt + 1)],
                            rhs=w4sb[:, 64 * b : 64 * (b + 1)],
                            start=True,
                            stop=True,
                        )
                    # --- dists = dists' + ||p||^2 ; min over k ; one-hot
                    dsb = workp.tile([128, CK], F32, tag="dsb")
                    nc.vector.tensor_tensor(dsb[:], dps[:], pnsb[:], op=add)
                    d3 = dsb.rearrange("p (c k) -> p c k", k=K)
                    minv = workp.tile([128, C], F32, tag="minv")
                    nc.vector.tensor_reduce(minv[:], d3, axis=ax_x, op=amin)
                    oh = workp.tile([128, CK], F32, tag="oh")
                    nc.vector.tensor_tensor(
                        oh.rearrange("p (c k) -> p c k", k=K),
                        d3,
                        minv[:, :, None].broadcast_to([128, C, K]),
                        op=iseq,
                    )
                    # --- transpose one-hot: [n, ck] -> [ck, n]
                    ot_ps = pt.tile([128, 256], F32, tag="otps")
                    for h in range(2):
                        nc.tensor.transpose(
                            ot_ps[:, 128 * h : 128 * (h + 1)],
                            oh[:, 128 * h : 128 * (h + 1)],
                            idsb[:],
                        )
                    ot = workp.tile([128, 256], F32, tag="ot")
                    nc.vector.tensor_copy(ot[:], ot_ps[:])
                    # --- LUT gather-accumulate: out[n, m] = onehot @ luts
                    ops = po.tile([128, M], F32, tag="ops")
                    for h in range(2):
                        for mh in range(2):
                            nc.tensor.matmul(
                                ops[:, 512 * mh : 512 * (mh + 1)],
                                lhsT=ot[:, 128 * h : 128 * (h + 1)],
                                rhs=lutsb[:, M * h + 512 * mh : M * h + 512 * (mh + 1)],
                                start=(h == 0),
                                stop=(h == 1),
                            )
                    osb = osbp.tile([128, M], F32, tag="osb")
                    # split PSUM->SBUF copy across ScalarE and VectorE
                    nc.scalar.copy(osb[:, :768], ops[:, :768])
                    nc.vector.tensor_copy(osb[:, 768:], ops[:, 768:])
                    nc.sync.dma_start(out[128 * t : 128 * (t + 1), :], osb[:])

    nc.compile()
    return nc


_CACHE = {}


def _prep_shared(protos: np.ndarray, luts: np.ndarray, mean: float):
    wfull = np.zeros((D, CK), dtype=np.float32)
    for c in range(C):
        wfull[S * c : S * (c + 1), K * c : K * (c + 1)] = -2.0 * protos[c].T
    w4h = np.ascontiguousarray(
        np.concatenate(
            [wfull[128 * b : 128 * (b + 1), 64 * b : 64 * (b + 1)] for b in range(4)],
            axis=1,
        )
    )
    pnorm = (protos.astype(np.float64) ** 2).sum(-1).astype(np.float32).reshape(1, CK)
    pnb = np.ascontiguousarray(np.broadcast_to(pnorm, (128, CK)))
    idm = np.eye(128, dtype=np.float32)
    lf = (luts.reshape(CK, M).astype(np.float64) * mean).astype(np.float32)
    luts2 = np.ascontiguousarray(
        np.concatenate([lf[:128], lf[128:]], axis=1)
    )  # [128, 2M]: col Mh+m = luts_flat[128h+p, m]
    return w4h, pnb, idm, luts2


def kernel(X: np.ndarray, protos: np.ndarray, luts: np.ndarray) -> np.ndarray:
    X = np.asarray(X, dtype=np.float32)
    protos = np.asarray(protos, dtype=np.float32)
    luts = np.asarray(luts, dtype=np.float32)

    mean = float(np.mean(X, dtype=np.float64))
    w4h, pnb, idm, luts2 = _prep_shared(protos, luts, mean)

    if "nc" not in _CACHE:
        _CACHE["nc"] = _build_program()
    nc = _CACHE["nc"]

    in_maps = []
    for i in range(NCORES):
        shard = X[NL * i : NL * (i + 1)]
        in_maps.append(
            {
                "xt": np.ascontiguousarray(shard.T),
                "w4": w4h,
                "pnb": pnb,
                "idm": idm,
                "luts": luts2,
            }
        )

    trace = bool(os.environ.get("VQ_TRACE"))
    if trace:
        trace = _install_profile_shim()

    res = run_bass_kernel_spmd(
        nc, in_maps, core_ids=list(range(NCORES)), trace=trace
    )
    _CACHE["exec_time_ns"] = res.exec_time_ns
    _CACHE["profile_json"] = res.profile_json

    return np.concatenate([res.results[i]["out"] for i in range(NCORES)], axis=0)


# revision 7
# speedup vs baseline: 1.5598x; 1.5598x over previous
"""MithralNN (PQ/vq_codebook) Trainium2 kernel.

out[n, m] = sum_c luts[c, argmin_k ||X[n, cS:(c+1)S] - protos[c,k]||^2, m] * mean(X)

Strategy (8 NeuronCores, data-parallel over rows of X):
  host:   transpose each core's X shard to [D, NL] (contiguous DMA of X^T),
          build block-diagonal -2*protos matmul weights, pre-broadcast
          ||p||^2 bias, pre-scale luts by mean(X).
  device: dists = X^T-block-matmuls + bias   (PE, contraction over d)
          one-hot(argmin_k) via min-reduce + is_equal   (DVE)
          onehot^T via PE transpose, out = onehot @ luts via PE matmuls
          PSUM->SBUF copies split across ScalarE/VectorE, DMA out.
"""

import os
import sys
import types

import numpy as np

import concourse.bacc as bacc
import concourse.mybir as mybir
import concourse.tile as tile
from concourse.bass_utils import run_bass_kernel_spmd

F32 = mybir.dt.float32
F32R = mybir.dt.float32r

N, D = 32768, 512
C, K, S = 16, 16, 32
CK = C * K          # 256
M = 1024
NCORES = 8
NL = N // NCORES    # 4096 rows per core
NT = NL // 128      # 32 row tiles per core


def _install_profile_shim():
    """Optionally enable NTFF profiling through the axon tunnel (dev only)."""
    try:
        if "antenv.axon_hooks" not in sys.modules:
            import antenv

            mod = types.ModuleType("antenv.axon_hooks")
            mod._hook = None
            mod.set_axon_ntff_profile_hook = lambda h: setattr(mod, "_hook", h)
            mod.get_axon_ntff_profile_hook = lambda: mod._hook
            sys.modules["antenv.axon_hooks"] = mod
            antenv.axon_hooks = mod
            from trn_agent_boot.trn_boot import _ntff_profile_via_ctypes

            mod.set_axon_ntff_profile_hook(
                _ntff_profile_via_ctypes("/opt/axon/libaxon_pjrt.so")
            )
        return True
    except Exception:
        return False


def _build_program():
    nc = bacc.Bacc("TRN2", target_bir_lowering=False, debug=False)

    xt = nc.dram_tensor("xt", [D, NL], F32, kind="ExternalInput")
    w4 = nc.dram_tensor("w4", [128, 256], F32, kind="ExternalInput")
    pnb = nc.dram_tensor("pnb", [128, CK], F32, kind="ExternalInput")
    idm = nc.dram_tensor("idm", [128, 128], F32R, kind="ExternalInput")
    luts = nc.dram_tensor("luts", [128, 2 * M], F32R, kind="ExternalInput")
    out = nc.dram_tensor("out", [NL, M], F32, kind="ExternalOutput")

    add = mybir.AluOpType.add
    amin = mybir.AluOpType.min
    iseq = mybir.AluOpType.is_equal
    ax_x = mybir.AxisListType.X

    with tile.TileContext(nc) as tc:
        with (
            tc.tile_pool(name="const", bufs=1) as constp,
            tc.tile_pool(name="xch", bufs=3) as xpool,
            tc.tile_pool(name="work", bufs=3) as workp,
            tc.tile_pool(name="osb", bufs=3) as osbp,
            tc.tile_pool(name="pdst", bufs=2, space="PSUM") as pd,
            tc.tile_pool(name="ptr", bufs=2, space="PSUM") as pt,
            tc.tile_pool(name="pout", bufs=2, space="PSUM") as po,
        ):
            w4sb = constp.tile([128, 256], F32)
            nc.sync.dma_start(w4sb[:], w4[:])
            pnsb = constp.tile([128, CK], F32)
            nc.sync.dma_start(pnsb[:], pnb[:])
            idsb = constp.tile([128, 128], F32R)
            nc.sync.dma_start(idsb[:], idm[:])
            lutsb = constp.tile([128, 2 * M], F32R)
            nc.sync.dma_start(lutsb[:], luts[:])

            for j in range(NT // 4):          # 8 chunks of 512 rows
                cj = xpool.tile([128, 4 * 512], F32, tag="xchunk")
                for b in range(4):
                    nc.sync.dma_start(
                        cj[:, 512 * b : 512 * (b + 1)],
                        xt[128 * b : 128 * (b + 1), 512 * j : 512 * (j + 1)],
                    )
                for tt in range(4):
                    t = 4 * j + tt
                    # --- encode: dists'[n, ck] = sum_d xt[d,n] * (-2W)[d,ck]
                    dps = pd.tile([128, CK], F32, tag="dps")
                    for b in range(4):
                        nc.tensor.matmul(
                            dps[:, 64 * b : 64 * (b + 1)],
                            lhsT=cj[:, 512 * b + 128 * tt : 512 * b + 128 * (tt + 1)],
                            rhs=w4sb[:, 64 * b : 64 * (b + 1)],
                            start=True,
                            stop=True,
                        )
                    # --- dists = dists' + ||p||^2 ; min over k ; one-hot
                    dsb = workp.tile([128, CK], F32, tag="dsb")
                    nc.vector.tensor_tensor(dsb[:], dps[:], pnsb[:], op=add)
                    d3 = dsb.rearrange("p (c k) -> p c k", k=K)
                    minv = workp.tile([128, C], F32, tag="minv")
                    nc.vector.tensor_reduce(minv[:], d3, axis=ax_x, op=amin)
                    oh = workp.tile([128, CK], F32R, tag="oh")
                    nc.vector.tensor_tensor(
                        oh.rearrange("p (c k) -> p c k", k=K),
                        d3,
                        minv[:, :, None].broadcast_to([128, C, K]),
                        op=iseq,
                    )
                    # --- transpose one-hot: [n, ck] -> [ck, n]
                    ot_ps = pt.tile([128, 256], F32R, tag="otps")
                    for h in range(2):
                        nc.tensor.transpose(
                            ot_ps[:, 128 * h : 128 * (h + 1)],
                            oh[:, 128 * h : 128 * (h + 1)],
                            idsb[:],
                        )
                    ot = workp.tile([128, 256], F32R, tag="ot")
                    nc.vector.tensor_copy(ot[:], ot_ps[:])
                    # --- LUT gather-accumulate: out[n, m] = onehot @ luts
                    ops = po.tile([128, M], F32, tag="ops")
                    for h in range(2):
                        for mh in range(2):
                            nc.tensor.matmul(
                                ops[:, 512 * mh : 512 * (mh + 1)],
                                lhsT=ot[:, 128 * h : 128 * (h + 1)],
                                rhs=lutsb[:, M * h + 512 * mh : M * h + 512 * (mh + 1)],
                                start=(h == 0),
                                stop=(h == 1),
                            )
                    osb = osbp.tile([128, M], F32, tag="osb")
                    # split PSUM->SBUF copy across ScalarE and VectorE
                    nc.scalar.copy(osb[:, :768], ops[:, :768])
                    nc.vector.tensor_copy(osb[:, 768:], ops[:, 768:])
                    nc.sync.dma_start(out[128 * t : 128 * (t + 1), :], osb[:])

    nc.compile()
    return nc


_CACHE = {}


def _prep_shared(protos: np.ndarray, luts: np.ndarray, mean: float):
    wfull = np.zeros((D, CK), dtype=np.float32)
    for c in range(C):
        wfull[S * c : S * (c + 1), K * c : K * (c + 1)] = -2.0 * protos[c].T
    w4h = np.ascontiguousarray(
        np.concatenate(
            [wfull[128 * b : 128 * (b + 1), 64 * b : 64 * (b + 1)] for b in range(4)],
            axis=1,
        )
    )
    pnorm = (protos.astype(np.float64) ** 2).sum(-1).astype(np.float32).reshape(1, CK)
    pnb = np.ascontiguousarray(np.broadcast_to(pnorm, (128, CK)))
    idm = np.eye(128, dtype=np.float32)
    lf = (luts.reshape(CK, M).astype(np.float64) * mean).astype(np.float32)
    luts2 = np.ascontiguousarray(
        np.concatenate([lf[:128], lf[128:]], axis=1)
    )  # [128, 2M]: col Mh+m = luts_flat[128h+p, m]
    return w4h, pnb, idm, luts2


def kernel(X: np.ndarray, protos: np.ndarray, luts: np.ndarray) -> np.ndarray:
    X = np.asarray(X, dtype=np.float32)
    protos = np.asarray(protos, dtype=np.float32)
    luts = np.asarray(luts, dtype=np.float32)

    mean = float(np.mean(X, dtype=np.float64))
    w4h, pnb, idm, luts2 = _prep_shared(protos, luts, mean)

    if "nc" not in _CACHE:
        _CACHE["nc"] = _build_program()
    nc = _CACHE["nc"]

    in_maps = []
    for i in range(NCORES):
        shard = X[NL * i : NL * (i + 1)]
        in_maps.append(
            {
                "xt": np.ascontiguousarray(shard.T),
                "w4": w4h,
                "pnb": pnb,
                "idm": idm,
                "luts": luts2,
            }
        )

    trace = bool(os.environ.get("VQ_TRACE"))
    if trace:
        trace = _install_profile_shim()

    res = run_bass_kernel_spmd(
        nc, in_maps, core_ids=list(range(NCORES)), trace=trace
    )
    _CACHE["exec_time_ns"] = res.exec_time_ns
    _CACHE["profile_json"] = res.profile_json

    return np.concatenate([res.results[i]["out"] for i in range(NCORES)], axis=0)
